# revision 12
# baseline (speedup 1.0000x reference)
"""LoRA MLP (gate_up + SiLU*up + down, each with rank-16 LoRA) on 8 TRN2 cores.

Strategy: pure data-parallel over tokens (16384 = 8 x 2048); weights are
replicated to every core, so no collectives are needed. The rank-16 LoRA is
merged into the base weights host-side (W_eff = W + A @ B, the standard
merged-adapter serving trick), so the device kernel is a plain dense MLP.
All matmul operands are bf16: full PE rate, and bf16 stationaries get fast
weight load so LDWEIGHTS hides completely under the 512-col matmuls (fp32r
weights cannot use FWL and leave ~180ns of exposed weight-load per matmul).
Activations stay transposed ([feature, token]) so every matmul consumes
natural-layout weights; accumulation is fp32 in PSUM.
"""

import numpy as np
import ml_dtypes

import concourse.mybir as mybir
import concourse.tile as tile
from concourse import bacc
from concourse.bass_utils import run_bass_kernel_spmd

TOKENS, D, FF, R = 16384, 1024, 2816, 16
N_CORES = 8
T_CORE = TOKENS // N_CORES  # 2048
TSUB = 512                  # psum free-dim tile (1 bank fp32)
DT = D // 128               # 8 d-model tiles
FFT = FF // 128             # 22 ff tiles
F32 = mybir.dt.float32
BF16 = mybir.dt.bfloat16
SILU = mybir.ActivationFunctionType.Silu
COPY = mybir.ActivationFunctionType.Copy

_prog_cache = {}


def _build():
    nc = bacc.Bacc("TRN2", target_bir_lowering=False, debug=False)
    xT = nc.dram_tensor("xT", [D, T_CORE], BF16, kind="ExternalInput").ap()
    w1 = nc.dram_tensor("W1", [D, 2 * FF], BF16, kind="ExternalInput").ap()
    w2 = nc.dram_tensor("W2", [FF, D], BF16, kind="ExternalInput").ap()
    out = nc.dram_tensor("out", [T_CORE, D], BF16, kind="ExternalOutput").ap()

    w1r = w1.rearrange("(dt p) f -> p dt f", p=128)   # [128, 8, 5632]
    w2r = w2.rearrange("(ft p) d -> p ft d", p=128)   # [128, 22, 1024]
    xTr = xT.rearrange("(dt p) t -> p dt t", p=128)   # [128, 8, 2048]

    with tile.TileContext(nc) as tc:
        with (
            tc.tile_pool(name="xp", bufs=1) as xp,
            tc.tile_pool(name="hp", bufs=1) as hp,
            tc.tile_pool(name="w1p", bufs=2) as w1p,
            tc.tile_pool(name="w2p", bufs=2) as w2p,
            tc.tile_pool(name="evp", bufs=2) as evp,
            tc.tile_pool(name="ps", bufs=1, space="PSUM") as ps,
        ):
            NPRE = 6  # i-tiles swept ts-outer while x streams in
            w1_tiles = {}

            def w1_dma(i):
                g = w1p.tile([128, DT, 128], BF16, tag="w1g", bufs=NPRE + 1)
                nc.sync.dma_start(g[:], w1r[:, :, i * 128 : (i + 1) * 128])
                u = w1p.tile([128, DT, 128], BF16, tag="w1u", bufs=NPRE + 1)
                nc.sync.dma_start(u[:], w1r[:, :, FF + i * 128 : FF + (i + 1) * 128])
                w1_tiles[i] = (g, u)

            # DMA issue order: first weight tile, then x chunk 0 (unblocks the
            # first chain ASAP), then the remaining pre-set weights interleaved
            # ahead of the later x chunks.
            xt_sb = xp.tile([128, DT, T_CORE], BF16, tag="xt")

            def xt_dma(ts):
                tsl = slice(ts * TSUB, (ts + 1) * TSUB)
                for d in range(DT):
                    nc.sync.dma_start(xt_sb[:, d, tsl], xTr[:, d, tsl])

            w1_dma(0)
            xt_dma(0)
            for i in range(1, NPRE):
                w1_dma(i)
            for ts in range(1, T_CORE // TSUB):
                xt_dma(ts)

            # Warm the PE clock gate (HAM) during the x DMA: scratch matmuls
            # on the already-loaded w1 tile, into a psum bank reused by po.
            pw = ps.tile([128, TSUB], F32, tag="po", bufs=2)
            g0 = w1_tiles[0][0]
            for _ in range(10):
                nc.tensor.matmul(
                    pw[:], g0[:, 0, :], g0[:, 0:4, :],
                    start=True, stop=True,
                )

            # ---- phase 1: h^T = silu(gate^T) * up^T ----
            h_sb = hp.tile([128, FFT, T_CORE], BF16, tag="h")

            def chains(i, ts):
                w1g, w1u = w1_tiles[i]
                tsl = slice(ts * TSUB, (ts + 1) * TSUB)
                pg = ps.tile([128, TSUB], F32, tag="pg", bufs=3)
                for d in range(DT):
                    nc.tensor.matmul(
                        pg[:], w1g[:, d, :], xt_sb[:, d, tsl],
                        start=(d == 0), stop=(d == DT - 1),
                    )
                pu = ps.tile([128, TSUB], F32, tag="pu", bufs=3)
                for d in range(DT):
                    nc.tensor.matmul(
                        pu[:], w1u[:, d, :], xt_sb[:, d, tsl],
                        start=(d == 0), stop=(d == DT - 1),
                    )
                tmp = evp.tile([128, TSUB], F32, tag="tmp")
                nc.scalar.activation(tmp[:], pg[:], SILU)
                nc.vector.tensor_mul(h_sb[:, i, tsl], tmp[:], pu[:])

            # ramp: sweep the pre-loaded i-tiles ts-outer so compute on x
            # chunk 0 hides the arrival of chunks 1..3
            for ts in range(T_CORE // TSUB):
                for i in range(NPRE):
                    chains(i, ts)
            # steady state: i-outer with one-ahead weight prefetch
            for i in range(NPRE, FFT):
                if i not in w1_tiles:
                    w1_dma(i)
                if i + 1 < FFT and i + 1 not in w1_tiles:
                    w1_dma(i + 1)
                for ts in range(T_CORE // TSUB):
                    chains(i, ts)

            # ---- phase 2: out = h^T.T @ W2 ----
            for dh in range(D // TSUB):
                dsl = slice(dh * TSUB, (dh + 1) * TSUB)
                w2_sb = w2p.tile([128, FFT, TSUB], BF16, tag="w2")
                for i in range(FFT):
                    nc.sync.dma_start(w2_sb[:, i, :], w2r[:, i, dsl])
                for tt in range(T_CORE // 128):
                    ttl = slice(tt * 128, (tt + 1) * 128)
                    po = ps.tile([128, TSUB], F32, tag="po", bufs=2)
                    for i in range(FFT):
                        nc.tensor.matmul(
                            po[:], h_sb[:, i, ttl], w2_sb[:, i, :],
                            start=(i == 0), stop=(i == FFT - 1),
                        )
                    o_sb = evp.tile([128, TSUB], BF16, tag="o")
                    nc.scalar.activation(o_sb[:], po[:], COPY)
                    nc.sync.dma_start(out[ttl, dsl], o_sb[:])
    nc.compile()
    return nc


def _get_prog():
    if "nc" not in _prog_cache:
        _prog_cache["nc"] = _build()
    return _prog_cache["nc"]


def run_sharded(inputs, trace=False):
    nc = _get_prog()
    bf16 = ml_dtypes.bfloat16
    x = np.asarray(inputs["x"], dtype=np.float32)
    # merge the rank-16 LoRA into the base weights (W_eff = W + A @ B)
    w1 = (
        np.asarray(inputs["W_gate_up"], dtype=np.float32)
        + np.asarray(inputs["A_gate_up"], dtype=np.float32)
        @ np.asarray(inputs["B_gate_up"], dtype=np.float32)
    ).astype(bf16)
    w2 = (
        np.asarray(inputs["W_down"], dtype=np.float32)
        + np.asarray(inputs["A_down"], dtype=np.float32)
        @ np.asarray(inputs["B_down"], dtype=np.float32)
    ).astype(bf16)
    weights = {"W1": np.ascontiguousarray(w1), "W2": np.ascontiguousarray(w2)}
    in_maps = []
    for c in range(N_CORES):
        xs = np.ascontiguousarray(x[c * T_CORE : (c + 1) * T_CORE].T.astype(bf16))
        in_maps.append({"xT": xs, **weights})
    res = run_bass_kernel_spmd(nc, in_maps, list(range(N_CORES)), trace=trace)
    outs = [np.asarray(res.results[c]["out"], dtype=np.float32) for c in range(N_CORES)]
    full = np.concatenate(outs, axis=0)
    return full, res


def kernel(**inputs):
    full, _ = run_sharded(inputs, trace=False)
    return full


# revision 13
# speedup vs baseline: 1.1917x; 1.1917x over previous
"""LoRA MLP (gate_up + SiLU*up + down, each with rank-16 LoRA) on 8 TRN2 cores.

Strategy: pure data-parallel over tokens (16384 = 8 x 2048); weights are
replicated to every core, so no collectives are needed. The rank-16 LoRA is
merged into the base weights host-side (W_eff = W + A @ B, the standard
merged-adapter serving trick), so the device kernel is a plain dense MLP.
All matmul operands are bf16: full PE rate, and bf16 stationaries get fast
weight load so LDWEIGHTS hides completely under the 512-col matmuls (fp32r
weights cannot use FWL and leave ~180ns of exposed weight-load per matmul).
Activations stay transposed ([feature, token]) so every matmul consumes
natural-layout weights; accumulation is fp32 in PSUM.
"""

import numpy as np
import ml_dtypes

import concourse.mybir as mybir
import concourse.tile as tile
from concourse import bacc
from concourse.bass_utils import run_bass_kernel_spmd

TOKENS, D, FF, R = 16384, 1024, 2816, 16
N_CORES = 8
T_CORE = TOKENS // N_CORES  # 2048
TSUB = 512                  # psum free-dim tile (1 bank fp32)
DT = D // 128               # 8 d-model tiles
FFT = FF // 128             # 22 ff tiles
F32 = mybir.dt.float32
BF16 = mybir.dt.bfloat16
SILU = mybir.ActivationFunctionType.Silu
COPY = mybir.ActivationFunctionType.Copy

_prog_cache = {}


def _build():
    nc = bacc.Bacc("TRN2", target_bir_lowering=False, debug=False)
    xT = nc.dram_tensor("xT", [D, T_CORE], BF16, kind="ExternalInput").ap()
    w1 = nc.dram_tensor("W1", [D, 2 * FF], BF16, kind="ExternalInput").ap()
    w2 = nc.dram_tensor("W2", [FF, D], BF16, kind="ExternalInput").ap()
    out = nc.dram_tensor("out", [T_CORE, D], BF16, kind="ExternalOutput").ap()

    w1r = w1.rearrange("(dt p) f -> p dt f", p=128)   # [128, 8, 5632]
    w2r = w2.rearrange("(ft p) d -> p ft d", p=128)   # [128, 22, 1024]
    xTr = xT.rearrange("(dt p) t -> p dt t", p=128)   # [128, 8, 2048]

    with tile.TileContext(nc) as tc:
        with (
            tc.tile_pool(name="xp", bufs=1) as xp,
            tc.tile_pool(name="hp", bufs=1) as hp,
            tc.tile_pool(name="w1p", bufs=2) as w1p,
            tc.tile_pool(name="w2p", bufs=2) as w2p,
            tc.tile_pool(name="evp", bufs=2) as evp,
            tc.tile_pool(name="ps", bufs=1, space="PSUM") as ps,
        ):
            NPRE = 6  # i-tiles swept ts-outer while x streams in
            w1_tiles = {}

            def w1_dma(i):
                g = w1p.tile([128, DT, 128], BF16, tag="w1g", bufs=NPRE + 1)
                nc.sync.dma_start(g[:], w1r[:, :, i * 128 : (i + 1) * 128])
                u = w1p.tile([128, DT, 128], BF16, tag="w1u", bufs=NPRE + 1)
                nc.sync.dma_start(u[:], w1r[:, :, FF + i * 128 : FF + (i + 1) * 128])
                w1_tiles[i] = (g, u)

            # DMA issue order: first weight tile, then x chunk 0 (unblocks the
            # first chain ASAP), then the remaining pre-set weights interleaved
            # ahead of the later x chunks.
            xt_sb = xp.tile([128, DT, T_CORE], BF16, tag="xt")

            def xt_dma(ts):
                tsl = slice(ts * TSUB, (ts + 1) * TSUB)
                for d in range(DT):
                    nc.sync.dma_start(xt_sb[:, d, tsl], xTr[:, d, tsl])

            w1_dma(0)
            xt_dma(0)
            for i in range(1, NPRE):
                w1_dma(i)
            for ts in range(1, T_CORE // TSUB):
                xt_dma(ts)

            # Warm the PE clock gate (HAM) during the x DMA: scratch matmuls
            # on the already-loaded w1 tile, into a psum bank reused by po.
            pw = ps.tile([128, TSUB], F32, tag="po", bufs=2)
            g0 = w1_tiles[0][0]
            for _ in range(7):
                nc.tensor.matmul(
                    pw[:], g0[:, 0, :], g0[:, 0:4, :],
                    start=True, stop=True,
                )

            # ---- phase 1: h^T = silu(gate^T) * up^T ----
            h_sb = hp.tile([128, FFT, T_CORE], BF16, tag="h")

            def chains(i, ts):
                w1g, w1u = w1_tiles[i]
                tsl = slice(ts * TSUB, (ts + 1) * TSUB)
                pg = ps.tile([128, TSUB], F32, tag="pg", bufs=3)
                for d in range(DT):
                    nc.tensor.matmul(
                        pg[:], w1g[:, d, :], xt_sb[:, d, tsl],
                        start=(d == 0), stop=(d == DT - 1),
                    )
                pu = ps.tile([128, TSUB], F32, tag="pu", bufs=3)
                for d in range(DT):
                    nc.tensor.matmul(
                        pu[:], w1u[:, d, :], xt_sb[:, d, tsl],
                        start=(d == 0), stop=(d == DT - 1),
                    )
                tmp = evp.tile([128, TSUB], F32, tag="tmp")
                nc.scalar.activation(tmp[:], pg[:], SILU)
                nc.vector.tensor_mul(h_sb[:, i, tsl], tmp[:], pu[:])

            # ramp: sweep the pre-loaded i-tiles ts-outer so compute on x
            # chunk 0 hides the arrival of chunks 1..3
            for ts in range(T_CORE // TSUB):
                for i in range(NPRE):
                    chains(i, ts)
            # steady state: i-outer with one-ahead weight prefetch
            for i in range(NPRE, FFT):
                if i not in w1_tiles:
                    w1_dma(i)
                if i + 1 < FFT and i + 1 not in w1_tiles:
                    w1_dma(i + 1)
                for ts in range(T_CORE // TSUB):
                    chains(i, ts)

            # ---- phase 2: out = h^T.T @ W2 ----
            for dh in range(D // TSUB):
                dsl = slice(dh * TSUB, (dh + 1) * TSUB)
                w2_sb = w2p.tile([128, FFT, TSUB], BF16, tag="w2")
                for i in range(FFT):
                    nc.sync.dma_start(w2_sb[:, i, :], w2r[:, i, dsl])
                for tt in range(T_CORE // 128):
                    ttl = slice(tt * 128, (tt + 1) * 128)
                    po = ps.tile([128, TSUB], F32, tag="po", bufs=2)
                    for i in range(FFT):
                        nc.tensor.matmul(
                            po[:], h_sb[:, i, ttl], w2_sb[:, i, :],
                            start=(i == 0), stop=(i == FFT - 1),
                        )
                    o_sb = evp.tile([128, TSUB], BF16, tag="o")
                    nc.scalar.activation(o_sb[:], po[:], COPY)
                    nc.sync.dma_start(out[ttl, dsl], o_sb[:])
    nc.compile()
    return nc


def _get_prog():
    if "nc" not in _prog_cache:
        _prog_cache["nc"] = _build()
    return _prog_cache["nc"]


def run_sharded(inputs, trace=False):
    nc = _get_prog()
    bf16 = ml_dtypes.bfloat16
    x = np.asarray(inputs["x"], dtype=np.float32)
    # merge the rank-16 LoRA into the base weights (W_eff = W + A @ B)
    w1 = (
        np.asarray(inputs["W_gate_up"], dtype=np.float32)
        + np.asarray(inputs["A_gate_up"], dtype=np.float32)
        @ np.asarray(inputs["B_gate_up"], dtype=np.float32)
    ).astype(bf16)
    w2 = (
        np.asarray(inputs["W_down"], dtype=np.float32)
        + np.asarray(inputs["A_down"], dtype=np.float32)
        @ np.asarray(inputs["B_down"], dtype=np.float32)
    ).astype(bf16)
    weights = {"W1": np.ascontiguousarray(w1), "W2": np.ascontiguousarray(w2)}
    in_maps = []
    for c in range(N_CORES):
        xs = np.ascontiguousarray(x[c * T_CORE : (c + 1) * T_CORE].T.astype(bf16))
        in_maps.append({"xT": xs, **weights})
    res = run_bass_kernel_spmd(nc, in_maps, list(range(N_CORES)), trace=trace)
    outs = [np.asarray(res.results[c]["out"], dtype=np.float32) for c in range(N_CORES)]
    full = np.concatenate(outs, axis=0)
    return full, res


def kernel(**inputs):
    full, _ = run_sharded(inputs, trace=False)
    return full
